# revision 1
# baseline (speedup 1.0000x reference)
"""Trainium2 Bass kernel for nn_Cat_Linear_Encoder (pairwise MLP edge decoder).

probs[i,j] = sigmoid(W2 @ relu(W1 @ cat(z_i, z_j) + b1) + b2) * (1 - eye)

Host-side factorization (all O(N*H), exact):
    A[i,h] = |W2_h| * (z_i @ Wa.T + b1)[h]      (Wa = W1[:, :D])
    B[j,h] = |W2_h| * (z_j @ Wb.T)[h]           (Wb = W1[:, D:])
    s_h    = sign(W2_h)
    adj[i,j] = sum_h s_h * relu(A[i,h] + B[j,h]) + b2
using w*relu(x) == sign(w)*relu(|w|*x).

Device (per core, i-shard of 256 rows = 128 i-pairs):
    - R tile [128, 2048]: partitions = (pair-parity x 64 h), free = j.
      Produced by DVE tensor_scalar (fused add+relu, bf16 4x mode), with
      a share offloaded to ACT (activation Relu with per-partition bias)
      to balance engine time (48:16 split).
    - PE reduces h (partition axis) with a sliding 2-column sparse weight
      window, 4-way column-group tiling (tile_position): 4 concurrent
      M=32 matmuls accumulate 4 different i-pairs into one PSUM bank.
    - ACT applies sigmoid PSUM->SBUF, DMA to DRAM.
Diagonal zeroing + shard concat happen on host.
"""

import numpy as np

N, D, H = 2048, 64, 64
NCORES = 8
SHARD = N // NCORES          # 256 i-rows per core
NPAIR = SHARD // 2           # 128 i-pairs per core
IBLK = SHARD // 128          # 2 psum row-blocks per core
JCH = 512                    # j-chunk = one PSUM bank of fp32
NJC = N // JCH               # 4

# R-producer engine weights (approx per-op cost in us) for load balancing.
# GpSimd tensor ops measured ~30us/op on HW (SBUF port contention with DVE)
# so it is excluded.
ENG_COST = {"V": 0.748, "A": 2.0}
# ACT also runs the sigmoid epilogue + table load (~3.5us per iblock), so its
# producer budget starts pre-charged.
ENG_INIT = {"V": 0.0, "A": 3.5}

_CACHE = {}
_prepared_in_maps = None


def _schedule_producers():
    """Greedy assignment of the 64 ips of one iblock to engines so each
    engine's total production time is balanced."""
    counts = dict(ENG_INIT)
    sched = []
    for idx in range(64):
        eng = min(ENG_COST, key=lambda e: counts[e] + ENG_COST[e])
        counts[eng] += ENG_COST[eng]
        sched.append(eng)
    return sched


def _build_bass(b2_val: float):
    import concourse.bacc as bacc
    import concourse.bass as bass
    import concourse.mybir as mybir
    from concourse.tile import TileContext

    bf16 = mybir.dt.bfloat16
    f32 = mybir.dt.float32

    nc = bacc.Bacc("TRN2", num_devices=NCORES)
    bdt_d = nc.dram_tensor("bdt", [128, N], bf16, kind="ExternalInput")
    ap_d = nc.dram_tensor("apairs", [128, NPAIR], f32, kind="ExternalInput")
    s_d = nc.dram_tensor("sbig", [128, 64], bf16, kind="ExternalInput")
    out_d = nc.dram_tensor("out", [SHARD, N], f32, kind="ExternalOutput")

    sched = _schedule_producers()

    with TileContext(nc) as tc:
        with (
            tc.tile_pool(name="const", bufs=1) as cpool,
            tc.tile_pool(name="r", bufs=16) as rpool,
            tc.tile_pool(name="o", bufs=4) as opool,
            tc.tile_pool(name="psum", bufs=8, space=bass.MemorySpace.PSUM) as ppool,
        ):
            bdt = cpool.tile([128, N], bf16, tag="bdt")
            apairs = cpool.tile([128, NPAIR], f32, tag="ap")
            sbig = cpool.tile([128, 64], bf16, tag="sbig")
            nc.sync.dma_start(out=apairs[:], in_=ap_d[:])
            nc.sync.dma_start(out=bdt[:], in_=bdt_d[:])
            nc.gpsimd.dma_start(out=sbig[:], in_=s_d[:])

            # dummy sigmoid on a scratch tile: front-loads the ACT table set
            # (includes relu) in parallel with the input DMAs
            warm = cpool.tile([128, 1], f32, tag="warm")
            nc.vector.memset(warm[:], 0.0)
            nc.scalar.activation(
                warm[:], warm[:], mybir.ActivationFunctionType.Sigmoid, bias=0.0
            )

            for ib in range(IBLK):
                ps = [
                    ppool.tile([128, JCH], f32, tag="ps", name=f"ps_{ib}_{jc}")
                    for jc in range(NJC)
                ]
                for l in range(16):
                    rtiles = []
                    for b in range(4):
                        ip = ib * 64 + 16 * b + l
                        r = rpool.tile([128, N], bf16, tag="r", name=f"r_{ip}")
                        eng = sched[16 * b + l]
                        if eng == "V":
                            nc.vector.tensor_scalar(
                                out=r[:],
                                in0=bdt[:],
                                scalar1=apairs[:, ip : ip + 1],
                                scalar2=0.0,
                                op0=mybir.AluOpType.add,
                                op1=mybir.AluOpType.max,
                            )
                        else:
                            nc.scalar.activation(
                                r[:],
                                bdt[:],
                                mybir.ActivationFunctionType.Relu,
                                bias=apairs[:, ip : ip + 1],
                                scale=1.0,
                            )
                        rtiles.append(r)
                    for jc in range(NJC):
                        for b in range(4):
                            nc.tensor.matmul(
                                ps[jc][32 * b : 32 * b + 32, :],
                                sbig[:, 32 - 2 * l : 64 - 2 * l],
                                rtiles[b][:, jc * JCH : (jc + 1) * JCH],
                                start=(l == 0),
                                stop=(l == 15),
                                tile_position=(0, 32 * b),
                            )
                for jc in range(NJC):
                    ot = opool.tile([128, JCH], f32, tag="ot", name=f"ot_{ib}_{jc}")
                    nc.scalar.activation(
                        ot[:],
                        ps[jc][:],
                        mybir.ActivationFunctionType.Sigmoid,
                        bias=float(b2_val),
                    )
                    nc.sync.dma_start(
                        out=out_d[ib * 128 : (ib + 1) * 128, jc * JCH : (jc + 1) * JCH],
                        in_=ot[:],
                    )
    nc.compile()
    return nc


def _default_inputs():
    """Regenerate reference setup_inputs() deterministically (CPU jax)."""
    import jax

    cpu = jax.devices("cpu")[0]
    with jax.default_device(cpu):
        key = jax.random.key(0)
        k0, k1, k2 = jax.random.split(key, 3)
        z = np.asarray(jax.random.normal(k0, (N, D), dtype="float32"))
        W1 = np.asarray(
            jax.random.normal(k1, (H, 2 * D), dtype="float32")
            * np.float32(1.0 / np.sqrt(2 * D))
        )
        b1 = np.zeros((H,), dtype=np.float32)
        W2 = np.asarray(
            jax.random.normal(k2, (1, H), dtype="float32")
            * np.float32(1.0 / np.sqrt(H))
        )
        b2 = np.zeros((1,), dtype=np.float32)
    return z, W1, b1, W2, b2


def kernel(z=None, W1=None, b1=None, W2=None, b2=None, **_unused):
    from concourse import bass_utils

    if any(x is None for x in (z, W1, b1, W2, b2)):
        dz, dW1, db1, dW2, db2 = _default_inputs()
        z = dz if z is None else np.asarray(z)
        W1 = dW1 if W1 is None else np.asarray(W1)
        b1 = db1 if b1 is None else np.asarray(b1)
        W2 = dW2 if W2 is None else np.asarray(W2)
        b2 = db2 if b2 is None else np.asarray(b2)
    z = np.asarray(z, np.float32)
    W1 = np.asarray(W1, np.float32)
    b1 = np.asarray(b1, np.float32)
    W2 = np.asarray(W2, np.float32)
    b2 = np.asarray(b2, np.float32)

    Wa, Wb = W1[:, :D], W1[:, D:]
    w2 = W2[0]                                     # [H]
    s = np.where(w2 >= 0, 1.0, -1.0).astype(np.float32)
    aw = np.abs(w2)
    A = (z @ Wa.T + b1[None, :]) * aw[None, :]     # [N, H]
    B = (z @ Wb.T) * aw[None, :]                   # [N, H]

    import ml_dtypes

    bdt = np.ascontiguousarray(
        np.concatenate([B.T, B.T], axis=0).astype(ml_dtypes.bfloat16)
    )  # [128, N]

    sbig = np.zeros((128, 64), dtype=ml_dtypes.bfloat16)
    sbig[0:64, 32] = s.astype(ml_dtypes.bfloat16)
    sbig[64:128, 33] = s.astype(ml_dtypes.bfloat16)

    # per-core A-pair columns: core c owns i in [c*SHARD, (c+1)*SHARD)
    in_maps = []
    for c in range(NCORES):
        Ash = A[c * SHARD : (c + 1) * SHARD]       # [256, H]
        ap = np.empty((128, NPAIR), dtype=np.float32)
        ap[0:64, :] = Ash[0::2].T                  # even rows of shard
        ap[64:128, :] = Ash[1::2].T                # odd rows
        in_maps.append(
            {
                "bdt": bdt,
                "apairs": np.ascontiguousarray(ap),
                "sbig": sbig,
            }
        )

    global _prepared_in_maps
    _prepared_in_maps = in_maps

    key = float(b2[0])
    if key not in _CACHE:
        _CACHE[key] = _build_bass(key)
    nc = _CACHE[key]

    res = bass_utils.run_bass_kernel_spmd(nc, in_maps, core_ids=list(range(NCORES)))
    probs = np.concatenate([r["out"] for r in res.results], axis=0)
    probs[np.arange(N), np.arange(N)] = 0.0
    return probs.astype(np.float32)


if __name__ == "__main__":
    out = kernel()
    print(out.shape, out.dtype, out[:3, :3])



# revision 3
# speedup vs baseline: 2.1435x; 2.1435x over previous
"""Trainium2 Bass kernel for nn_Cat_Linear_Encoder (pairwise MLP edge decoder).

probs[i,j] = sigmoid(W2 @ relu(W1 @ cat(z_i, z_j) + b1) + b2) * (1 - eye)

Quantized-interpolation formulation. With Wa = W1[:, :D], Wb = W1[:, D:],
s_h = sign(W2_h), scaled features
    A[i,h] = |W2_h| * (z_i @ Wa.T + b1)[h],   B[j,h] = |W2_h| * (z_j @ Wb.T)[h],
the pre-sigmoid logit is  adj[i,j] = sum_h s_h relu(A_ih + B_jh) + b2.

For each h, A[:, h] is bracketed on a uniform node grid v_h (c_h nodes,
sum_h c_h = K ~ 1152).  relu(x + B_jh) is piecewise linear in x except at
the kink x = -B_jh, so replacing A_ih by linear interpolation between its
two bracketing nodes is EXACT unless the kink lands inside that bin
(error <= bin/4, halved to bin/8 by a minimax shift of the two
kink-bracketing table entries).  This turns the whole computation into a
dense matmul:
    adj ~= E @ T,   E[i,(h,q)] = 2-hot interp weights (host-built),
                    T[(h,q),j] = s_h relu(v_hq + B_jh)  (host-built).

Device (per core, 256-row i-shard): a [256, K] x [K, 2048] bf16 matmul on
the PE (K-chunked PSUM accumulation), sigmoid epilogue on ACT, pipelined
against the T-table DMA (j-major order).  No elementwise production at
all -- the relu lives inside the host-built table.

Host: feature/table construction is O(N*K); diagonal zeroing + shard
concat as before.
"""

import numpy as np

N, D, H = 2048, 64, 64
NCORES = 8
SHARD = N // NCORES          # 256 i-rows per core
KC = 9                       # K chunks of 128 -> K2 = 1152
K2 = KC * 128
JCH = 512                    # j-chunk = one PSUM bank of fp32
NJC = N // JCH               # 4
NWARM = 9                    # dummy matmuls to warm the PE clock (HAM)

_CACHE = {}
_prepared_in_maps = None


def _build_bass(b2_val: float):
    import concourse.bacc as bacc
    import concourse.bass as bass
    import concourse.mybir as mybir
    from concourse.tile import TileContext

    bf16 = mybir.dt.bfloat16
    f32 = mybir.dt.float32

    nc = bacc.Bacc("TRN2", num_devices=NCORES)
    et_d = nc.dram_tensor("et", [128, KC * 2 * 128], bf16, kind="ExternalInput")
    t_d = nc.dram_tensor("t", [K2, N], bf16, kind="ExternalInput")
    out_d = nc.dram_tensor("out", [SHARD, N], f32, kind="ExternalOutput")

    dma_engines = None  # set inside context

    with TileContext(nc) as tc:
        with (
            tc.tile_pool(name="const", bufs=1) as cpool,
            tc.tile_pool(name="t", bufs=KC * NJC) as tpool,
            tc.tile_pool(name="o", bufs=4) as opool,
            tc.tile_pool(name="psum", bufs=5, space=bass.MemorySpace.PSUM) as ppool,
            tc.tile_pool(name="pwarm", bufs=1, space=bass.MemorySpace.PSUM) as wpool,
        ):
            # --- prologue: weights DMA, ACT table preload, PE warmup ---
            etall = cpool.tile([128, KC * 2 * 128], bf16, tag="et")
            nc.sync.dma_start(out=etall[:], in_=et_d[:])

            warm = cpool.tile([128, 1], f32, tag="warm")
            nc.vector.memset(warm[:], 0.0)
            nc.scalar.activation(
                warm[:], warm[:], mybir.ActivationFunctionType.Sigmoid, bias=0.0
            )

            # dummy matmuls: keep the PE busy >3.4us before the real work so
            # the HAM clock-gate opens (1.2 -> 2.4 GHz)
            wsrc = cpool.tile([128, JCH], bf16, tag="wsrc")
            nc.vector.memset(wsrc[:], 0.0)
            wps = wpool.tile([128, JCH], f32, tag="wps")
            for w in range(NWARM):
                nc.tensor.matmul(
                    wps[:], wsrc[:, 0:128], wsrc[:], start=(w == 0), stop=(w == NWARM - 1)
                )

            # --- T-table DMA, j-major so PE can start after the first jc ---
            ttiles = {}
            qs = [nc.sync, nc.gpsimd]
            qi = 0
            for jc in range(NJC):
                for kc in range(KC):
                    t_tile = tpool.tile([128, JCH], bf16, tag="t", name=f"t_{jc}_{kc}")
                    qs[qi % len(qs)].dma_start(
                        out=t_tile[:],
                        in_=t_d[kc * 128 : (kc + 1) * 128, jc * JCH : (jc + 1) * JCH],
                    )
                    qi += 1
                    ttiles[(jc, kc)] = t_tile

            # --- main: K-accumulated matmul per (rb, jc), sigmoid, store ---
            for jc in range(NJC):
                ps = [
                    ppool.tile([128, JCH], f32, tag="ps", name=f"ps_{jc}_{rb}")
                    for rb in range(2)
                ]
                for kc in range(KC):
                    for rb in range(2):
                        col = (kc * 2 + rb) * 128
                        nc.tensor.matmul(
                            ps[rb][:],
                            etall[:, col : col + 128],
                            ttiles[(jc, kc)][:],
                            start=(kc == 0),
                            stop=(kc == KC - 1),
                        )
                for rb in range(2):
                    ot = opool.tile([128, JCH], f32, tag="ot", name=f"ot_{jc}_{rb}")
                    nc.scalar.activation(
                        ot[:],
                        ps[rb][:],
                        mybir.ActivationFunctionType.Sigmoid,
                        bias=float(b2_val),
                    )
                    (nc.sync if rb == 0 else nc.gpsimd).dma_start(
                        out=out_d[rb * 128 : (rb + 1) * 128, jc * JCH : (jc + 1) * JCH],
                        in_=ot[:],
                    )
    nc.compile()
    return nc


def _build_tables(z, W1, b1, W2):
    """Host-side construction of E [N, K2] and T [K2, N] (float32)."""
    Wa, Wb = W1[:, :D], W1[:, D:]
    w2 = W2[0]
    s = np.where(w2 >= 0, 1.0, -1.0).astype(np.float32)
    aw = np.abs(w2)
    A = (z @ Wa.T + b1[None, :]) * aw[None, :]   # [N, H] scaled
    B = (z @ Wb.T) * aw[None, :]                 # [N, H] scaled

    lo = A.min(axis=0)
    hi = A.max(axis=0)
    rng = np.maximum(hi - lo, 1e-6)

    # choose per-h node counts: uniform absolute bin width delta, total <= K2
    def total(delta):
        return int(np.maximum(2, np.ceil(rng / delta).astype(int) + 1).sum())

    d_lo, d_hi = rng.sum() / (4 * K2), rng.sum()
    for _ in range(60):
        mid = 0.5 * (d_lo + d_hi)
        if total(mid) > K2:
            d_lo = mid
        else:
            d_hi = mid
    counts = np.maximum(2, np.ceil(rng / d_hi).astype(int) + 1)
    # spend any remaining budget on the h's with the widest bins
    while counts.sum() < K2:
        width = rng / (counts - 1)
        counts[np.argmax(width)] += 1
    assert counts.sum() == K2, counts.sum()

    E = np.zeros((N, K2), dtype=np.float32)
    T = np.zeros((K2, N), dtype=np.float32)
    off = 0
    rows = np.arange(N)
    for h in range(H):
        c = int(counts[h])
        v = np.linspace(lo[h], hi[h], c).astype(np.float32)
        Th = s[h] * np.maximum(v[:, None] + B[None, :, h], 0.0)   # [c, N]
        # minimax shift: halve the kink-bin secant error
        t = -B[:, h]
        inside = (t > v[0]) & (t < v[-1])
        jdx = np.clip(np.searchsorted(v, t, side="right") - 1, 0, c - 2)
        dv = v[jdx + 1] - v[jdx]
        g = np.where(inside, (v[jdx + 1] - t) * (t - v[jdx]) / dv, 0.0).astype(
            np.float32
        )
        Th[jdx, rows] -= s[h] * g / 2
        Th[jdx + 1, rows] -= s[h] * g / 2
        T[off : off + c, :] = Th

        idx = np.clip(np.searchsorted(v, A[:, h], side="right") - 1, 0, c - 2)
        lam = np.clip((A[:, h] - v[idx]) / (v[idx + 1] - v[idx]), 0.0, 1.0)
        E[rows, off + idx] = 1.0 - lam
        E[rows, off + idx + 1] = lam
        off += c
    return E, T


def _default_inputs():
    """Regenerate reference setup_inputs() deterministically (CPU jax)."""
    import jax

    cpu = jax.devices("cpu")[0]
    with jax.default_device(cpu):
        key = jax.random.key(0)
        k0, k1, k2 = jax.random.split(key, 3)
        z = np.asarray(jax.random.normal(k0, (N, D), dtype="float32"))
        W1 = np.asarray(
            jax.random.normal(k1, (H, 2 * D), dtype="float32")
            * np.float32(1.0 / np.sqrt(2 * D))
        )
        b1 = np.zeros((H,), dtype=np.float32)
        W2 = np.asarray(
            jax.random.normal(k2, (1, H), dtype="float32")
            * np.float32(1.0 / np.sqrt(H))
        )
        b2 = np.zeros((1,), dtype=np.float32)
    return z, W1, b1, W2, b2


def kernel(z=None, W1=None, b1=None, W2=None, b2=None, **_unused):
    from concourse import bass_utils
    import ml_dtypes

    if any(x is None for x in (z, W1, b1, W2, b2)):
        dz, dW1, db1, dW2, db2 = _default_inputs()
        z = dz if z is None else np.asarray(z)
        W1 = dW1 if W1 is None else np.asarray(W1)
        b1 = db1 if b1 is None else np.asarray(b1)
        W2 = dW2 if W2 is None else np.asarray(W2)
        b2 = db2 if b2 is None else np.asarray(b2)
    z = np.asarray(z, np.float32)
    W1 = np.asarray(W1, np.float32)
    b1 = np.asarray(b1, np.float32)
    W2 = np.asarray(W2, np.float32)
    b2 = np.asarray(b2, np.float32)

    E, T = _build_tables(z, W1, b1, W2)
    t_in = np.ascontiguousarray(T.astype(ml_dtypes.bfloat16))

    in_maps = []
    for c in range(NCORES):
        Ec = E[c * SHARD : (c + 1) * SHARD]                  # [256, K2]
        # stationary layout: row p holds Et chunks for each (kc, rb):
        # et[p, (kc*2+rb)*128 + i] = Ec[rb*128+i, kc*128+p]
        X = Ec.reshape(2, 128, KC, 128)                      # [rb, i, kc, p]
        et = np.ascontiguousarray(
            X.transpose(3, 2, 0, 1).reshape(128, KC * 2 * 128).astype(
                ml_dtypes.bfloat16
            )
        )
        in_maps.append({"et": et, "t": t_in})

    global _prepared_in_maps
    _prepared_in_maps = in_maps

    key = float(b2[0])
    if key not in _CACHE:
        _CACHE[key] = _build_bass(key)
    nc = _CACHE[key]

    res = bass_utils.run_bass_kernel_spmd(nc, in_maps, core_ids=list(range(NCORES)))
    probs = np.concatenate([r["out"] for r in res.results], axis=0)
    probs[np.arange(N), np.arange(N)] = 0.0
    return probs.astype(np.float32)


if __name__ == "__main__":
    out = kernel()
    print(out.shape, out.dtype, out[:3, :3])
